# revision 8
# baseline (speedup 1.0000x reference)
"""Trainium2 Bass kernel for nn_BatchedQNodeLayer (8-qubit batched QNode).

Math: for an RX-angle-embedded product state pushed through a fixed
(theta-dependent) 2-layer strongly-entangling circuit and measured with
<Z_0>, the output is

    out_b = 0.5 + 0.5 * <psi(x_b)| M(theta) |psi(x_b)>

M expanded in the {I,Y,Z}^8 Pauli basis (X terms vanish for RX product
states) gives out_b as a multilinear form in per-wire features
[1, -sin(x_w), cos(x_w)].  The coefficient tensor factors hierarchically
(operator-Schmidt ranks are tiny for a shallow circuit; K=R1=R2=2 here),
reducing the per-element device work to ~130 elementwise MACs over
sin/cos planes.  All coefficients are computed on the host from theta
(O(1) in batch) and baked into the instruction stream as immediates.

Layout per core: batch shard of 16384 elements as [128 partitions, 128
free] planes; sin/cos via the ACT engine (range-reduced to [-pi, pi]
with the fp32 magic-rounding trick since the Sin table is only accurate
there); products/MAC-chains on the vector engine.
"""

import sys

sys.path.insert(0, "/opt/trn_rl_repo")

import numpy as np

N_QUBITS = 8
DIM = 256
N_CORES = 8
B_TOTAL = 131072
B_CORE = B_TOTAL // N_CORES  # 16384
P = 128                      # partitions
J = B_CORE // P              # 128 free elems per partition

TWO_PI = float(2.0 * np.pi)
INV_2PI = float(1.0 / (2.0 * np.pi))
MAGIC = float(1.5 * 2**23)   # fp32 round-to-nearest-integer bias
HALF_PI = float(np.pi / 2.0)


# ----------------------------------------------------------------------------
# Host-side precompute: theta -> hierarchical factor tensors
# ----------------------------------------------------------------------------

def _evolved_observable(theta):
    """M = U^dag Z0 U as dense 256x256 complex128 (numpy only)."""
    def rot(phi, th, om):
        c, s = np.cos(th / 2), np.sin(th / 2)
        return np.array([
            [np.exp(-0.5j * (phi + om)) * c, -np.exp(0.5j * (phi - om)) * s],
            [np.exp(-0.5j * (phi - om)) * s, np.exp(0.5j * (phi + om)) * c]])

    U = np.eye(DIM, dtype=np.complex128)

    def apply_1q(U, g, w):
        Ur = U.reshape([2] * N_QUBITS + [DIM])
        Ur = np.moveaxis(Ur, w, 0)
        Ur = np.tensordot(g, Ur, axes=([1], [0]))
        Ur = np.moveaxis(Ur, 0, w)
        return Ur.reshape(DIM, DIM)

    def apply_cnot(U, c, t):
        rows = np.arange(DIM)
        cbit = (rows >> (N_QUBITS - 1 - c)) & 1
        perm = np.where(cbit == 1, rows ^ (1 << (N_QUBITS - 1 - t)), rows)
        return U[perm, :]

    for l in range(2):
        for w in range(N_QUBITS):
            U = apply_1q(U, rot(*theta[l, w]), w)
        r = (l % (N_QUBITS - 1)) + 1
        for w in range(N_QUBITS):
            U = apply_cnot(U, w, (w + r) % N_QUBITS)
    z0 = 1.0 - 2.0 * ((np.arange(DIM) >> (N_QUBITS - 1)) & 1)
    return U.conj().T @ (z0[:, None] * U)


def _iyz_tensor(M):
    """Pauli coefficients over {I,Y,Z}^8 (axis order I,Y,Z per wire)."""
    I2 = np.eye(2, dtype=np.complex128)
    X = np.array([[0, 1], [1, 0]], dtype=np.complex128)
    Y = np.array([[0, -1j], [1j, 0]], dtype=np.complex128)
    Z = np.array([[1, 0], [0, -1]], dtype=np.complex128)
    T = M.reshape([2] * 16)
    perm = []
    for w in range(N_QUBITS):
        perm += [w, 8 + w]
    T = np.transpose(T, perm).reshape([4] * N_QUBITS)
    A = np.zeros((4, 4), dtype=np.complex128)
    for p, Pm in enumerate([I2, X, Y, Z]):
        A[p] = (Pm.T / 2).reshape(-1)
    for w in range(N_QUBITS):
        T = np.moveaxis(np.tensordot(A, T, axes=([1], [w])), 0, w)
    C = T.real
    idx = [0, 2, 3]
    return C[np.ix_(idx, idx, idx, idx, idx, idx, idx, idx)].copy()


def _factorize(theta, tol=1e-9):
    M = _evolved_observable(np.asarray(theta, np.float64))
    C = _iyz_tensor(M) * 0.5  # folds out = 0.5 + 0.5*ev
    S = C.reshape(81, 81)
    U, s, Vt = np.linalg.svd(S)
    K = max(1, int((s > s[0] * tol).sum()))
    A = U[:, :K] * np.sqrt(s[:K])
    Bv = Vt[:K].T * np.sqrt(s[:K])
    AL = A.reshape(9, 9, K)
    M1 = AL.reshape(9, 9 * K)
    P1, t1, Q1t = np.linalg.svd(M1, full_matrices=False)
    R1 = max(1, int((t1 > t1[0] * tol).sum()))
    W01 = P1[:, :R1] * np.sqrt(t1[:R1])                                  # [9,R1]
    V23 = Q1t[:R1].reshape(R1, 9, K) * np.sqrt(t1[:R1])[:, None, None]   # [R1,9,K]
    BR = Bv.reshape(9, 9, K).transpose(1, 0, 2)
    M2 = BR.reshape(9, 9 * K)
    P2, t2, Q2t = np.linalg.svd(M2, full_matrices=False)
    R2 = max(1, int((t2 > t2[0] * tol).sum()))
    W67 = P2[:, :R2] * np.sqrt(t2[:R2])                                  # [9,R2]
    V45 = Q2t[:R2].reshape(R2, 9, K) * np.sqrt(t2[:R2])[:, None, None]   # [R2,9,K]
    return dict(K=K, R1=R1, R2=R2, W01=W01, V23=V23, W67=W67, V45=V45)


# ----------------------------------------------------------------------------
# Bass program
# ----------------------------------------------------------------------------

def _build_program(F):
    from concourse import bass, mybir, tile
    from concourse.vector_clock import ScopedClock

    class SafeTileContext(tile.TileContext):
        """This walrus rejects instructions carrying more than one sync
        wait.  After scheduling, park every extra wait on a same-engine
        nop inserted immediately before the instruction."""

        def schedule_and_allocate(self):
            ret = super().schedule_and_allocate()
            nc = self.nc
            for bb in list(nc.main_func.blocks):
                i = 0
                while i < len(bb.instructions):
                    ins = bb.instructions[i]
                    si = ins.sync_info
                    waits = list(si.on_wait or []) if si else []
                    lim = 1
                    if len(waits) > lim:
                        ins.sync_info = mybir.SyncInfo(
                            on_wait=waits[:lim], on_update=si.on_update)
                        rest = waits[lim:]
                        nops = []
                        while rest:
                            n = nc.engines[ins.engine].nop()
                            n.ins.sync_info = mybir.SyncInfo(
                                on_wait=rest[:1], on_update=[])
                            rest = rest[1:]
                            nops.append(n.ins)
                        for n in nops:
                            for blk in nc.main_func.blocks:
                                if n in blk.instructions:
                                    blk.instructions.remove(n)
                                    break
                        bb.instructions[i:i] = nops
                        i += len(nops)
                    i += 1
            return ret

    f32 = mybir.dt.float32
    OP = mybir.AluOpType
    AF = mybir.ActivationFunctionType

    nc = bass.Bass()
    x_in = nc.dram_tensor("x", [B_CORE, N_QUBITS], f32, kind="ExternalInput")
    y_out = nc.dram_tensor("out", [B_CORE, 1], f32, kind="ExternalOutput")

    with SafeTileContext(nc) as tc:
        with tc.tile_pool(name="pool", bufs=1) as pool:
            X = pool.tile([P, J * N_QUBITS], f32)        # (p, j*8+w)
            T1 = pool.tile([P, J * N_QUBITS], f32)
            Y = pool.tile([P, N_QUBITS * J], f32)        # w-major (p, w*128+j)
            # TRIG = [ sin block (w-major 1024) | cos block (1024) ]
            TRIG = pool.tile([P, 2 * N_QUBITS * J], f32)
            AB = pool.tile([P, N_QUBITS * J], f32)
            # PROD cols = (pair, a, b, j): a/b = 0:sin,1:cos of wA/wB
            PROD = pool.tile([P, 16 * J], f32)
            hp = pool.tile([P, 1], f32)

            nc.vector.memset(hp[:, :], HALF_PI)

            # input DMA, queue-parallel chunks, issued from the idle SP engine
            xv = x_in.rearrange("(p j) w -> p (j w)", p=P)
            for c in range(8):
                nc.sync.dma_start(X[16 * c:16 * (c + 1), :],
                                  xv[16 * c:16 * (c + 1), :])

            # range reduction: y = x - 2pi*round(x/(2pi)), reordered w-major
            nc.vector.tensor_scalar(T1[:, :], X[:, :], INV_2PI, MAGIC,
                                    OP.mult, OP.add)
            nc.vector.tensor_scalar(T1[:, :], T1[:, :], MAGIC, None,
                                    OP.subtract)
            # out free iter (w, j): Y[:, w*J+j] = T1[:, j*8+w]*(-2pi) + X[:, j*8+w]
            Yv = Y[:, :].rearrange("p (w j) -> p w j", w=N_QUBITS)
            T1v = T1[:, :].rearrange("p (j w) -> p w j", w=N_QUBITS)
            Xv = X[:, :].rearrange("p (j w) -> p w j", w=N_QUBITS)
            nc.vector.scalar_tensor_tensor(Yv, T1v, -TWO_PI, Xv,
                                           OP.mult, OP.add)

            # trig: sin(y); cos(y) = sin(pi/2 - |y|)
            SIN = TRIG[:, 0:N_QUBITS * J]
            COS = TRIG[:, N_QUBITS * J:2 * N_QUBITS * J]
            nc.scalar.activation(SIN, Y[:, :], AF.Sin)
            nc.scalar.activation(AB[:, :], Y[:, :], AF.Abs)
            nc.scalar.activation(COS, AB[:, :], AF.Sin,
                                 bias=hp[:, :], scale=-1.0)

            def Sw(w):
                return TRIG[:, w * J:(w + 1) * J]

            def Cw(w):
                return TRIG[:, (N_QUBITS + w) * J:(N_QUBITS + w + 1) * J]

            # all 16 pair products in two wide-AP ops (3 free dims max):
            # PROD[p, pr, a, b, j] = TRIG[p, a, 2pr, j] * TRIG[p, b, 2pr+1, j]
            tv = TRIG[:, :].rearrange("p (a pr t j) -> p a pr t j",
                                      a=2, pr=4, t=2)
            ov = PROD[:, :].rearrange("p (pr a b j) -> p pr a b j",
                                      pr=4, a=2, b=2)
            in2 = tv[:, :, :, 1:2, :].transpose([0, 2, 1, 3, 4]) \
                .squeeze(3)                     # [p, pr, b, j], b-stride 1024
            for a in range(2):
                in1 = tv[:, a:a + 1, :, 0:1, :].squeeze(1) \
                    .broadcast_to([P, 4, 2, J])  # [p, pr, b(bcast), j]
                out_a = ov[:, :, a:a + 1, :, :].squeeze(2)
                nc.gpsimd.tensor_tensor(out_a, in1, in2, OP.mult)

            def prod(pair_idx, a, b):
                base = (pair_idx * 4 + a * 2 + b) * J
                return PROD[:, base:base + J]

            PAIR_IDX = {(0, 1): 0, (2, 3): 1, (4, 5): 2, (6, 7): 3}

            def chain(eng, name, pair, w9):
                """q = sum_a w9[a] * mono_a over pair; returns the q tile.

                mono index a = 3*iA+iB with per-wire features [1,-s,c]."""
                wA, wB = pair
                pi = PAIR_IDX[pair]
                terms = [
                    (Sw(wB), -w9[1]), (Cw(wB), w9[2]),
                    (Sw(wA), -w9[3]), (Cw(wA), w9[6]),
                    (prod(pi, 0, 0), w9[4]), (prod(pi, 0, 1), -w9[5]),
                    (prod(pi, 1, 0), -w9[7]), (prod(pi, 1, 1), w9[8]),
                ]
                terms = [(ap, c) for (ap, c) in terms if abs(c) > 1e-12]
                q = pool.tile([P, J], f32, tag=name)
                if not terms:
                    eng.memset(q[:, :], float(w9[0]))
                    return q
                ap0, c0 = terms[0]
                eng.tensor_scalar(q[:, :], ap0, float(c0), float(w9[0]),
                                  OP.mult, OP.add)
                for (ap, c) in terms[1:]:
                    eng.scalar_tensor_tensor(q[:, :], ap, float(c),
                                             q[:, :], OP.mult, OP.add)
                return q

            K, R1, R2 = F["K"], F["R1"], F["R2"]
            V = nc.vector
            G = nc.gpsimd
            u01 = [chain(V, f"u01_{m}", (0, 1), F["W01"][:, m])
                   for m in range(R1)]
            u67 = [chain(V, f"u67_{m}", (6, 7), F["W67"][:, m])
                   for m in range(R2)]
            v23 = [[chain(V, f"v23_{m}_{k}", (2, 3), F["V23"][m, :, k])
                    for k in range(K)] for m in range(R1)]
            v45 = [[chain(V, f"v45_{m}_{k}", (4, 5), F["V45"][m, :, k])
                    for k in range(K)] for m in range(R2)]

            def combine(eng, name, us, vs):
                """sum_m us[m]*vs[m]"""
                acc = pool.tile([P, J], f32, tag=name)
                eng.tensor_mul(acc[:, :], us[0][:, :], vs[0][:, :])
                for m in range(1, len(us)):
                    t = pool.tile([P, J], f32, tag=name + "t")
                    eng.tensor_mul(t[:, :], us[m][:, :], vs[m][:, :])
                    eng.tensor_add(acc[:, :], acc[:, :], t[:, :])
                return acc

            uL = [combine(V, f"uL{k}", u01, [v23[m][k] for m in range(R1)])
                  for k in range(K)]
            uR = [combine(G, f"uR{k}", u67, [v45[m][k] for m in range(R2)])
                  for k in range(K)]

            acc = pool.tile([P, J], f32)
            nc.vector.tensor_mul(acc[:, :], uL[0][:, :], uR[0][:, :])
            for k in range(1, K):
                t = pool.tile([P, J], f32, tag="topt")
                nc.vector.tensor_mul(t[:, :], uL[k][:, :], uR[k][:, :])
                nc.vector.tensor_add(acc[:, :], acc[:, :], t[:, :])
            OUT = pool.tile([P, J], f32)
            nc.vector.tensor_scalar(OUT[:, :], acc[:, :], 1.0, 0.5,
                                    OP.mult, OP.add)

            yv = y_out.rearrange("(p j) o -> p (j o)", p=P)
            for c in range(4):
                nc.sync.dma_start(yv[32 * c:32 * (c + 1), :],
                                  OUT[32 * c:32 * (c + 1), :])
    return nc


_PROGRAM_CACHE = {}
LAST_RESULT = None


def kernel(x: np.ndarray, theta: np.ndarray) -> np.ndarray:
    import os
    from concourse.bass_utils import run_bass_kernel_spmd

    x = np.ascontiguousarray(np.asarray(x, dtype=np.float32))
    theta = np.asarray(theta, dtype=np.float32)
    assert x.shape == (B_TOTAL, N_QUBITS), x.shape

    key = theta.tobytes()
    nc = _PROGRAM_CACHE.get(key)
    if nc is None:
        F = _factorize(theta)
        nc = _build_program(F)
        _PROGRAM_CACHE[key] = nc

    shards = [x[i * B_CORE:(i + 1) * B_CORE] for i in range(N_CORES)]
    in_maps = [{"x": s} for s in shards]
    trace = bool(int(os.environ.get("KERNEL_PROFILE", "0")))
    res = run_bass_kernel_spmd(nc, in_maps, list(range(N_CORES)), trace=trace)
    global LAST_RESULT
    LAST_RESULT = res
    out = np.concatenate([res.results[i]["out"] for i in range(N_CORES)], axis=0)
    return out.astype(np.float32, copy=False)


# revision 11
# speedup vs baseline: 1.2663x; 1.2663x over previous
"""Trainium2 Bass kernel for nn_BatchedQNodeLayer (8-qubit batched QNode).

Math: for an RX-angle-embedded product state pushed through a fixed
(theta-dependent) 2-layer strongly-entangling circuit and measured with
<Z_0>, the output is

    out_b = 0.5 + 0.5 * <psi(x_b)| M(theta) |psi(x_b)>

M expanded in the {I,Y,Z}^8 Pauli basis (X terms vanish for RX product
states) gives out_b as a multilinear form in per-wire features
[1, -sin(x_w), cos(x_w)].  The coefficient tensor factors hierarchically
(operator-Schmidt ranks are tiny for a shallow circuit; K=R1=R2=2 here),
reducing the per-element device work to ~130 elementwise MACs over
sin/cos planes.  All coefficients are computed on the host from theta
(O(1) in batch) and baked into the instruction stream as immediates.

Layout per core: batch shard of 16384 elements as [128 partitions, 128
free] planes; sin/cos via the ACT engine (range-reduced to [-pi, pi]
with the fp32 magic-rounding trick since the Sin table is only accurate
there); products/MAC-chains on the vector engine.
"""

import sys

sys.path.insert(0, "/opt/trn_rl_repo")

import numpy as np

N_QUBITS = 8
DIM = 256
N_CORES = 8
B_TOTAL = 131072
B_CORE = B_TOTAL // N_CORES  # 16384
P = 128                      # partitions
J = B_CORE // P              # 128 free elems per partition

TWO_PI = float(2.0 * np.pi)
INV_2PI = float(1.0 / (2.0 * np.pi))
MAGIC = float(1.5 * 2**23)   # fp32 round-to-nearest-integer bias
HALF_PI = float(np.pi / 2.0)


# ----------------------------------------------------------------------------
# Host-side precompute: theta -> hierarchical factor tensors
# ----------------------------------------------------------------------------

def _evolved_observable(theta):
    """M = U^dag Z0 U as dense 256x256 complex128 (numpy only)."""
    def rot(phi, th, om):
        c, s = np.cos(th / 2), np.sin(th / 2)
        return np.array([
            [np.exp(-0.5j * (phi + om)) * c, -np.exp(0.5j * (phi - om)) * s],
            [np.exp(-0.5j * (phi - om)) * s, np.exp(0.5j * (phi + om)) * c]])

    U = np.eye(DIM, dtype=np.complex128)

    def apply_1q(U, g, w):
        Ur = U.reshape([2] * N_QUBITS + [DIM])
        Ur = np.moveaxis(Ur, w, 0)
        Ur = np.tensordot(g, Ur, axes=([1], [0]))
        Ur = np.moveaxis(Ur, 0, w)
        return Ur.reshape(DIM, DIM)

    def apply_cnot(U, c, t):
        rows = np.arange(DIM)
        cbit = (rows >> (N_QUBITS - 1 - c)) & 1
        perm = np.where(cbit == 1, rows ^ (1 << (N_QUBITS - 1 - t)), rows)
        return U[perm, :]

    for l in range(2):
        for w in range(N_QUBITS):
            U = apply_1q(U, rot(*theta[l, w]), w)
        r = (l % (N_QUBITS - 1)) + 1
        for w in range(N_QUBITS):
            U = apply_cnot(U, w, (w + r) % N_QUBITS)
    z0 = 1.0 - 2.0 * ((np.arange(DIM) >> (N_QUBITS - 1)) & 1)
    return U.conj().T @ (z0[:, None] * U)


def _iyz_tensor(M):
    """Pauli coefficients over {I,Y,Z}^8 (axis order I,Y,Z per wire)."""
    I2 = np.eye(2, dtype=np.complex128)
    X = np.array([[0, 1], [1, 0]], dtype=np.complex128)
    Y = np.array([[0, -1j], [1j, 0]], dtype=np.complex128)
    Z = np.array([[1, 0], [0, -1]], dtype=np.complex128)
    T = M.reshape([2] * 16)
    perm = []
    for w in range(N_QUBITS):
        perm += [w, 8 + w]
    T = np.transpose(T, perm).reshape([4] * N_QUBITS)
    A = np.zeros((4, 4), dtype=np.complex128)
    for p, Pm in enumerate([I2, X, Y, Z]):
        A[p] = (Pm.T / 2).reshape(-1)
    for w in range(N_QUBITS):
        T = np.moveaxis(np.tensordot(A, T, axes=([1], [w])), 0, w)
    C = T.real
    idx = [0, 2, 3]
    return C[np.ix_(idx, idx, idx, idx, idx, idx, idx, idx)].copy()


def _factorize(theta, tol=1e-9):
    M = _evolved_observable(np.asarray(theta, np.float64))
    C = _iyz_tensor(M) * 0.5  # folds out = 0.5 + 0.5*ev
    S = C.reshape(81, 81)
    U, s, Vt = np.linalg.svd(S)
    K = max(1, int((s > s[0] * tol).sum()))
    A = U[:, :K] * np.sqrt(s[:K])
    Bv = Vt[:K].T * np.sqrt(s[:K])
    AL = A.reshape(9, 9, K)
    M1 = AL.reshape(9, 9 * K)
    P1, t1, Q1t = np.linalg.svd(M1, full_matrices=False)
    R1 = max(1, int((t1 > t1[0] * tol).sum()))
    W01 = P1[:, :R1] * np.sqrt(t1[:R1])                                  # [9,R1]
    V23 = Q1t[:R1].reshape(R1, 9, K) * np.sqrt(t1[:R1])[:, None, None]   # [R1,9,K]
    BR = Bv.reshape(9, 9, K).transpose(1, 0, 2)
    M2 = BR.reshape(9, 9 * K)
    P2, t2, Q2t = np.linalg.svd(M2, full_matrices=False)
    R2 = max(1, int((t2 > t2[0] * tol).sum()))
    W67 = P2[:, :R2] * np.sqrt(t2[:R2])                                  # [9,R2]
    V45 = Q2t[:R2].reshape(R2, 9, K) * np.sqrt(t2[:R2])[:, None, None]   # [R2,9,K]
    return dict(K=K, R1=R1, R2=R2, W01=W01, V23=V23, W67=W67, V45=V45)


# ----------------------------------------------------------------------------
# Bass program
# ----------------------------------------------------------------------------

def _build_program(F):
    from concourse import bass, mybir, tile
    from concourse.vector_clock import ScopedClock

    class SafeTileContext(tile.TileContext):
        """This walrus rejects instructions carrying more than one sync
        wait.  After scheduling, park every extra wait on a same-engine
        nop inserted immediately before the instruction."""

        def schedule_and_allocate(self):
            ret = super().schedule_and_allocate()
            nc = self.nc
            for bb in list(nc.main_func.blocks):
                i = 0
                while i < len(bb.instructions):
                    ins = bb.instructions[i]
                    si = ins.sync_info
                    waits = list(si.on_wait or []) if si else []
                    lim = 1
                    if len(waits) > lim:
                        ins.sync_info = mybir.SyncInfo(
                            on_wait=waits[:lim], on_update=si.on_update)
                        rest = waits[lim:]
                        nops = []
                        while rest:
                            n = nc.engines[ins.engine].nop()
                            n.ins.sync_info = mybir.SyncInfo(
                                on_wait=rest[:1], on_update=[])
                            rest = rest[1:]
                            nops.append(n.ins)
                        for n in nops:
                            for blk in nc.main_func.blocks:
                                if n in blk.instructions:
                                    blk.instructions.remove(n)
                                    break
                        bb.instructions[i:i] = nops
                        i += len(nops)
                    i += 1
            return ret

    f32 = mybir.dt.float32
    OP = mybir.AluOpType
    AF = mybir.ActivationFunctionType

    nc = bass.Bass()
    x_in = nc.dram_tensor("x", [B_CORE, N_QUBITS], f32, kind="ExternalInput")
    y_out = nc.dram_tensor("out", [B_CORE, 1], f32, kind="ExternalOutput")

    with SafeTileContext(nc) as tc:
        with tc.tile_pool(name="pool", bufs=1) as pool:
            X = pool.tile([P, J * N_QUBITS], f32)        # (p, j*8+w)
            T1 = pool.tile([P, J * N_QUBITS], f32)
            Y = pool.tile([P, N_QUBITS * J], f32)        # w-major (p, w*128+j)
            # TRIG = [ sin block (w-major 1024) | cos block (1024) ]
            TRIG = pool.tile([P, 2 * N_QUBITS * J], f32)
            AB = pool.tile([P, N_QUBITS * J], f32)
            # PROD cols = (pair, a, b, j): a/b = 0:sin,1:cos of wA/wB
            PROD = pool.tile([P, 16 * J], f32)
            hp = pool.tile([P, 1], f32)

            nc.vector.memset(hp[:, :], HALF_PI)

            # Preload the ACT Sin table while input DMA runs: a tiny dummy
            # activation on the (memset) hp tile.
            warm = pool.tile([P, 1], f32)
            nc.scalar.activation(warm[:, :], hp[:, :], AF.Sin)

            # input DMA, chunks spread across engines so the 64KiB direct
            # copies run in parallel instead of serializing on one engine
            xv = x_in.rearrange("(p j) w -> p (j w)", p=P)
            dma_engines = [nc.sync, nc.gpsimd, nc.scalar]
            for c in range(8):
                dma_engines[c % 3].dma_start(X[16 * c:16 * (c + 1), :],
                                             xv[16 * c:16 * (c + 1), :])

            # range reduction: y = x - 2pi*round(x/(2pi)), reordered w-major
            nc.vector.tensor_scalar(T1[:, :], X[:, :], INV_2PI, MAGIC,
                                    OP.mult, OP.add)
            nc.vector.tensor_scalar(T1[:, :], T1[:, :], MAGIC, None,
                                    OP.subtract)
            # per-wire: Y[:, w*J+j] = T1[:, j*8+w]*(-2pi) + X[:, j*8+w]
            # (separate [128,J] ops beat one big strided op on DVE)
            for w in range(N_QUBITS):
                Yw = Y[:, w * J:(w + 1) * J]
                T1w = T1[:, :].rearrange("p (j w) -> p w j",
                                         w=N_QUBITS)[:, w, :]
                Xw = X[:, :].rearrange("p (j w) -> p w j",
                                       w=N_QUBITS)[:, w, :]
                nc.vector.scalar_tensor_tensor(Yw, T1w, -TWO_PI, Xw,
                                               OP.mult, OP.add)

            # trig: sin(y); cos(y) = sin(pi/2 - |y|)
            SIN = TRIG[:, 0:N_QUBITS * J]
            COS = TRIG[:, N_QUBITS * J:2 * N_QUBITS * J]
            nc.scalar.activation(SIN, Y[:, :], AF.Sin)
            nc.scalar.activation(AB[:, :], Y[:, :], AF.Abs)
            nc.scalar.activation(COS, AB[:, :], AF.Sin,
                                 bias=hp[:, :], scale=-1.0)

            def Sw(w):
                return TRIG[:, w * J:(w + 1) * J]

            def Cw(w):
                return TRIG[:, (N_QUBITS + w) * J:(N_QUBITS + w + 1) * J]

            # all 16 pair products in two wide-AP ops (3 free dims max):
            # PROD[p, pr, a, b, j] = TRIG[p, a, 2pr, j] * TRIG[p, b, 2pr+1, j]
            tv = TRIG[:, :].rearrange("p (a pr t j) -> p a pr t j",
                                      a=2, pr=4, t=2)
            ov = PROD[:, :].rearrange("p (pr a b j) -> p pr a b j",
                                      pr=4, a=2, b=2)
            in2 = tv[:, :, :, 1:2, :].transpose([0, 2, 1, 3, 4]) \
                .squeeze(3)                     # [p, pr, b, j], b-stride 1024
            for a in range(2):
                in1 = tv[:, a:a + 1, :, 0:1, :].squeeze(1) \
                    .broadcast_to([P, 4, 2, J])  # [p, pr, b(bcast), j]
                out_a = ov[:, :, a:a + 1, :, :].squeeze(2)
                nc.vector.tensor_tensor(out_a, in1, in2, OP.mult)

            def prod(pair_idx, a, b):
                base = (pair_idx * 4 + a * 2 + b) * J
                return PROD[:, base:base + J]

            PAIR_IDX = {(0, 1): 0, (2, 3): 1, (4, 5): 2, (6, 7): 3}

            def chain(eng, name, pair, w9):
                """q = sum_a w9[a] * mono_a over pair; returns the q tile.

                mono index a = 3*iA+iB with per-wire features [1,-s,c]."""
                wA, wB = pair
                pi = PAIR_IDX[pair]
                terms = [
                    (Sw(wB), -w9[1]), (Cw(wB), w9[2]),
                    (Sw(wA), -w9[3]), (Cw(wA), w9[6]),
                    (prod(pi, 0, 0), w9[4]), (prod(pi, 0, 1), -w9[5]),
                    (prod(pi, 1, 0), -w9[7]), (prod(pi, 1, 1), w9[8]),
                ]
                terms = [(ap, c) for (ap, c) in terms if abs(c) > 1e-12]
                q = pool.tile([P, J], f32, tag=name)
                if not terms:
                    eng.memset(q[:, :], float(w9[0]))
                    return q
                ap0, c0 = terms[0]
                nc.scalar.activation(q[:, :], ap0, AF.Copy,
                                     bias=float(w9[0]), scale=float(c0))
                for (ap, c) in terms[1:]:
                    eng.scalar_tensor_tensor(q[:, :], ap, float(c),
                                             q[:, :], OP.mult, OP.add)
                return q

            K, R1, R2 = F["K"], F["R1"], F["R2"]
            V = nc.vector
            G = nc.gpsimd
            u01 = [chain(V, f"u01_{m}", (0, 1), F["W01"][:, m])
                   for m in range(R1)]
            u67 = [chain(V, f"u67_{m}", (6, 7), F["W67"][:, m])
                   for m in range(R2)]
            v23 = [[chain(V, f"v23_{m}_{k}", (2, 3), F["V23"][m, :, k])
                    for k in range(K)] for m in range(R1)]
            v45 = [[chain(V, f"v45_{m}_{k}", (4, 5), F["V45"][m, :, k])
                    for k in range(K)] for m in range(R2)]

            def combine(eng, name, us, vs):
                """sum_m us[m]*vs[m]"""
                acc = pool.tile([P, J], f32, tag=name)
                eng.tensor_mul(acc[:, :], us[0][:, :], vs[0][:, :])
                for m in range(1, len(us)):
                    t = pool.tile([P, J], f32, tag=name + "t")
                    eng.tensor_mul(t[:, :], us[m][:, :], vs[m][:, :])
                    eng.tensor_add(acc[:, :], acc[:, :], t[:, :])
                return acc

            uL = [combine(V, f"uL{k}", u01, [v23[m][k] for m in range(R1)])
                  for k in range(K)]
            uR = [combine(V, f"uR{k}", u67, [v45[m][k] for m in range(R2)])
                  for k in range(K)]

            acc = pool.tile([P, J], f32)
            nc.vector.tensor_mul(acc[:, :], uL[0][:, :], uR[0][:, :])
            for k in range(1, K):
                t = pool.tile([P, J], f32, tag="topt")
                nc.vector.tensor_mul(t[:, :], uL[k][:, :], uR[k][:, :])
                nc.vector.tensor_add(acc[:, :], acc[:, :], t[:, :])
            OUT = pool.tile([P, J], f32)
            nc.scalar.activation(OUT[:, :], acc[:, :], AF.Copy,
                                 bias=0.5, scale=1.0)

            yv = y_out.rearrange("(p j) o -> p (j o)", p=P)
            out_engines = [nc.sync, nc.gpsimd, nc.scalar, nc.sync]
            for c in range(4):
                out_engines[c].dma_start(yv[32 * c:32 * (c + 1), :],
                                         OUT[32 * c:32 * (c + 1), :])
    return nc


_PROGRAM_CACHE = {}
LAST_RESULT = None


def kernel(x: np.ndarray, theta: np.ndarray) -> np.ndarray:
    import os
    from concourse.bass_utils import run_bass_kernel_spmd

    x = np.ascontiguousarray(np.asarray(x, dtype=np.float32))
    theta = np.asarray(theta, dtype=np.float32)
    assert x.shape == (B_TOTAL, N_QUBITS), x.shape

    key = theta.tobytes()
    nc = _PROGRAM_CACHE.get(key)
    if nc is None:
        F = _factorize(theta)
        nc = _build_program(F)
        _PROGRAM_CACHE[key] = nc

    shards = [x[i * B_CORE:(i + 1) * B_CORE] for i in range(N_CORES)]
    in_maps = [{"x": s} for s in shards]
    trace = bool(int(os.environ.get("KERNEL_PROFILE", "0")))
    res = run_bass_kernel_spmd(nc, in_maps, list(range(N_CORES)), trace=trace)
    global LAST_RESULT
    LAST_RESULT = res
    out = np.concatenate([res.results[i]["out"] for i in range(N_CORES)], axis=0)
    return out.astype(np.float32, copy=False)


# revision 14
# speedup vs baseline: 1.6218x; 1.2808x over previous
"""Trainium2 Bass kernel for nn_BatchedQNodeLayer (8-qubit batched QNode).

Math: for an RX-angle-embedded product state pushed through a fixed
(theta-dependent) 2-layer strongly-entangling circuit and measured with
<Z_0>, the output is

    out_b = 0.5 + 0.5 * <psi(x_b)| M(theta) |psi(x_b)>

M expanded in the {I,Y,Z}^8 Pauli basis (X terms vanish for RX product
states) gives out_b as a multilinear form in per-wire features
[1, -sin(x_w), cos(x_w)].  The coefficient tensor factors hierarchically
(operator-Schmidt ranks are tiny for a shallow circuit; K=R1=R2=2 here),
reducing the per-element device work to ~130 elementwise MACs over
sin/cos planes.  All coefficients are computed on the host from theta
(O(1) in batch) and baked into the instruction stream as immediates.

Layout per core: batch shard of 16384 elements as [128 partitions, 128
free] planes; sin/cos via the ACT engine (range-reduced to [-pi, pi]
with the fp32 magic-rounding trick since the Sin table is only accurate
there); products/MAC-chains on the vector engine.
"""

import sys

sys.path.insert(0, "/opt/trn_rl_repo")

import numpy as np

N_QUBITS = 8
DIM = 256
N_CORES = 8
B_TOTAL = 131072
B_CORE = B_TOTAL // N_CORES  # 16384
P = 128                      # partitions
J = B_CORE // P              # 128 free elems per partition

TWO_PI = float(2.0 * np.pi)
INV_2PI = float(1.0 / (2.0 * np.pi))
MAGIC = float(1.5 * 2**23)   # fp32 round-to-nearest-integer bias
HALF_PI = float(np.pi / 2.0)


# ----------------------------------------------------------------------------
# Host-side precompute: theta -> hierarchical factor tensors
# ----------------------------------------------------------------------------

def _evolved_observable(theta):
    """M = U^dag Z0 U as dense 256x256 complex128 (numpy only)."""
    def rot(phi, th, om):
        c, s = np.cos(th / 2), np.sin(th / 2)
        return np.array([
            [np.exp(-0.5j * (phi + om)) * c, -np.exp(0.5j * (phi - om)) * s],
            [np.exp(-0.5j * (phi - om)) * s, np.exp(0.5j * (phi + om)) * c]])

    U = np.eye(DIM, dtype=np.complex128)

    def apply_1q(U, g, w):
        Ur = U.reshape([2] * N_QUBITS + [DIM])
        Ur = np.moveaxis(Ur, w, 0)
        Ur = np.tensordot(g, Ur, axes=([1], [0]))
        Ur = np.moveaxis(Ur, 0, w)
        return Ur.reshape(DIM, DIM)

    def apply_cnot(U, c, t):
        rows = np.arange(DIM)
        cbit = (rows >> (N_QUBITS - 1 - c)) & 1
        perm = np.where(cbit == 1, rows ^ (1 << (N_QUBITS - 1 - t)), rows)
        return U[perm, :]

    for l in range(2):
        for w in range(N_QUBITS):
            U = apply_1q(U, rot(*theta[l, w]), w)
        r = (l % (N_QUBITS - 1)) + 1
        for w in range(N_QUBITS):
            U = apply_cnot(U, w, (w + r) % N_QUBITS)
    z0 = 1.0 - 2.0 * ((np.arange(DIM) >> (N_QUBITS - 1)) & 1)
    return U.conj().T @ (z0[:, None] * U)


def _iyz_tensor(M):
    """Pauli coefficients over {I,Y,Z}^8 (axis order I,Y,Z per wire)."""
    I2 = np.eye(2, dtype=np.complex128)
    X = np.array([[0, 1], [1, 0]], dtype=np.complex128)
    Y = np.array([[0, -1j], [1j, 0]], dtype=np.complex128)
    Z = np.array([[1, 0], [0, -1]], dtype=np.complex128)
    T = M.reshape([2] * 16)
    perm = []
    for w in range(N_QUBITS):
        perm += [w, 8 + w]
    T = np.transpose(T, perm).reshape([4] * N_QUBITS)
    A = np.zeros((4, 4), dtype=np.complex128)
    for p, Pm in enumerate([I2, X, Y, Z]):
        A[p] = (Pm.T / 2).reshape(-1)
    for w in range(N_QUBITS):
        T = np.moveaxis(np.tensordot(A, T, axes=([1], [w])), 0, w)
    C = T.real
    idx = [0, 2, 3]
    return C[np.ix_(idx, idx, idx, idx, idx, idx, idx, idx)].copy()


def _factorize(theta, tol=1e-9):
    M = _evolved_observable(np.asarray(theta, np.float64))
    C = _iyz_tensor(M) * 0.5  # folds out = 0.5 + 0.5*ev
    S = C.reshape(81, 81)
    U, s, Vt = np.linalg.svd(S)
    K = max(1, int((s > s[0] * tol).sum()))
    A = U[:, :K] * np.sqrt(s[:K])
    Bv = Vt[:K].T * np.sqrt(s[:K])
    AL = A.reshape(9, 9, K)
    M1 = AL.reshape(9, 9 * K)
    P1, t1, Q1t = np.linalg.svd(M1, full_matrices=False)
    R1 = max(1, int((t1 > t1[0] * tol).sum()))
    W01 = P1[:, :R1] * np.sqrt(t1[:R1])                                  # [9,R1]
    V23 = Q1t[:R1].reshape(R1, 9, K) * np.sqrt(t1[:R1])[:, None, None]   # [R1,9,K]
    BR = Bv.reshape(9, 9, K).transpose(1, 0, 2)
    M2 = BR.reshape(9, 9 * K)
    P2, t2, Q2t = np.linalg.svd(M2, full_matrices=False)
    R2 = max(1, int((t2 > t2[0] * tol).sum()))
    W67 = P2[:, :R2] * np.sqrt(t2[:R2])                                  # [9,R2]
    V45 = Q2t[:R2].reshape(R2, 9, K) * np.sqrt(t2[:R2])[:, None, None]   # [R2,9,K]
    return dict(K=K, R1=R1, R2=R2, W01=W01, V23=V23, W67=W67, V45=V45)


def _prune_err(F, thr):
    """Max deviation of thr-pruned factors vs full, on random inputs."""
    rng = np.random.default_rng(0)
    x = rng.standard_normal((4096, N_QUBITS))
    sin, cos = np.sin(x), np.cos(x)

    def feats(wA, wB):
        SA, CA = sin[:, wA], cos[:, wA]
        SB, CB = sin[:, wB], cos[:, wB]
        one = np.ones_like(SA)
        return np.stack([one, -SB, CB, -SA, SA * SB, -SA * CB,
                         CA, -CA * SB, CA * CB], 1)

    f01, f23 = feats(0, 1), feats(2, 3)
    f45, f67 = feats(4, 5), feats(6, 7)

    def ev(W01, V23, W67, V45):
        u01 = f01 @ W01
        v23 = np.einsum('ba,mak->bmk', f23, V23)
        u67 = f67 @ W67
        v45 = np.einsum('bc,mck->bmk', f45, V45)
        uLk = np.einsum('bm,bmk->bk', u01, v23)
        uRk = np.einsum('bm,bmk->bk', u67, v45)
        return (uLk * uRk).sum(1)

    full = ev(F["W01"], F["V23"], F["W67"], F["V45"])
    pr = [np.where(np.abs(F[k]) > thr, F[k], 0.0)
          for k in ("W01", "V23", "W67", "V45")]
    return float(np.abs(full - ev(*pr)).max())


def _pick_prune_thr(F):
    for thr in (1e-5, 1e-6, 1e-7, 0.0):
        if _prune_err(F, thr) < 3e-5:
            return thr
    return 0.0


# ----------------------------------------------------------------------------
# Bass program
# ----------------------------------------------------------------------------

def _build_program(F, prune_thr=1e-5):
    from concourse import bass, mybir, tile
    from concourse.vector_clock import ScopedClock

    class SafeTileContext(tile.TileContext):
        """This walrus rejects instructions carrying more than one sync
        wait.  After scheduling, park every extra wait on a same-engine
        nop inserted immediately before the instruction."""

        def schedule_and_allocate(self):
            ret = super().schedule_and_allocate()
            nc = self.nc
            for bb in list(nc.main_func.blocks):
                i = 0
                while i < len(bb.instructions):
                    ins = bb.instructions[i]
                    si = ins.sync_info
                    waits = list(si.on_wait or []) if si else []
                    lim = 1
                    if len(waits) > lim:
                        ins.sync_info = mybir.SyncInfo(
                            on_wait=waits[:lim], on_update=si.on_update)
                        rest = waits[lim:]
                        nops = []
                        while rest:
                            n = nc.engines[ins.engine].nop()
                            n.ins.sync_info = mybir.SyncInfo(
                                on_wait=rest[:1], on_update=[])
                            rest = rest[1:]
                            nops.append(n.ins)
                        for n in nops:
                            for blk in nc.main_func.blocks:
                                if n in blk.instructions:
                                    blk.instructions.remove(n)
                                    break
                        bb.instructions[i:i] = nops
                        i += len(nops)
                    i += 1
            return ret

    f32 = mybir.dt.float32
    OP = mybir.AluOpType
    AF = mybir.ActivationFunctionType

    nc = bass.Bass()
    x_in = nc.dram_tensor("x", [B_CORE, N_QUBITS], f32, kind="ExternalInput")
    y_out = nc.dram_tensor("out", [B_CORE, 1], f32, kind="ExternalOutput")

    with SafeTileContext(nc) as tc:
        with tc.tile_pool(name="pool", bufs=1) as pool:
            X = pool.tile([P, J * N_QUBITS], f32)        # (p, j*8+w)
            T1 = pool.tile([P, J * N_QUBITS], f32)
            Y = pool.tile([P, N_QUBITS * J], f32)        # w-major (p, w*128+j)
            # TRIG = [ sin block (w-major 1024) | cos block (1024) ]
            TRIG = pool.tile([P, 2 * N_QUBITS * J], f32)
            AB = pool.tile([P, N_QUBITS * J], f32)
            # PROD cols = (pair, a, b, j): a/b = 0:sin,1:cos of wA/wB
            PROD = pool.tile([P, 16 * J], f32)
            hp = pool.tile([P, 1], f32)

            nc.vector.memset(hp[:, :], HALF_PI)

            # Preload the ACT Sin table while input DMA runs: a tiny dummy
            # activation on the (memset) hp tile.
            warm = pool.tile([P, 1], f32)
            nc.scalar.activation(warm[:, :], hp[:, :], AF.Sin)

            # input DMA, chunks spread across engines so the 64KiB direct
            # copies run in parallel instead of serializing on one engine
            xv = x_in.rearrange("(p j) w -> p (j w)", p=P)
            dma_engines = [nc.sync, nc.gpsimd, nc.scalar]
            for c in range(8):
                dma_engines[c % 3].dma_start(X[16 * c:16 * (c + 1), :],
                                             xv[16 * c:16 * (c + 1), :])

            # range reduction: y = x - 2pi*round(x/(2pi)), reordered w-major
            nc.vector.tensor_scalar(T1[:, :], X[:, :], INV_2PI, MAGIC,
                                    OP.mult, OP.add)
            nc.vector.tensor_scalar(T1[:, :], T1[:, :], MAGIC, None,
                                    OP.subtract)
            # per-wire: Y[:, w*J+j] = T1[:, j*8+w]*(-2pi) + X[:, j*8+w]
            # (separate [128,J] ops beat one big strided op on DVE)
            for w in range(N_QUBITS):
                Yw = Y[:, w * J:(w + 1) * J]
                T1w = T1[:, :].rearrange("p (j w) -> p w j",
                                         w=N_QUBITS)[:, w, :]
                Xw = X[:, :].rearrange("p (j w) -> p w j",
                                       w=N_QUBITS)[:, w, :]
                nc.vector.scalar_tensor_tensor(Yw, T1w, -TWO_PI, Xw,
                                               OP.mult, OP.add)

            # trig in halves (wires 0-3 then 4-7) so left-pair work starts
            # while the right half is still on the ACT engine:
            # sin(y); cos(y) = sin(pi/2 - |y|)
            SIN = TRIG[:, 0:N_QUBITS * J]
            COS = TRIG[:, N_QUBITS * J:2 * N_QUBITS * J]
            H = 4 * J
            for h in range(2):
                sl = slice(h * H, (h + 1) * H)
                nc.scalar.activation(SIN[:, sl], Y[:, sl], AF.Sin)
                nc.scalar.activation(AB[:, sl], Y[:, sl], AF.Abs)
                nc.scalar.activation(COS[:, sl], AB[:, sl], AF.Sin,
                                     bias=hp[:, :], scale=-1.0)

            def Sw(w):
                return TRIG[:, w * J:(w + 1) * J]

            def Cw(w):
                return TRIG[:, (N_QUBITS + w) * J:(N_QUBITS + w + 1) * J]

            # all 16 pair products in four wide-AP ops (3 free dims max,
            # split by half so they chase the trig halves):
            # PROD[p, pr, a, b, j] = TRIG[p, a, 2pr, j] * TRIG[p, b, 2pr+1, j]
            tv = TRIG[:, :].rearrange("p (a pr t j) -> p a pr t j",
                                      a=2, pr=4, t=2)
            ov = PROD[:, :].rearrange("p (pr a b j) -> p pr a b j",
                                      pr=4, a=2, b=2)
            in2 = tv[:, :, :, 1:2, :].transpose([0, 2, 1, 3, 4]) \
                .squeeze(3)                     # [p, pr, b, j], b-stride 1024
            for h in range(2):
                pr = slice(2 * h, 2 * h + 2)
                for a in range(2):
                    in1 = tv[:, a:a + 1, pr, 0:1, :].squeeze(1) \
                        .broadcast_to([P, 2, 2, J])  # [p, pr, b(bcast), j]
                    out_a = ov[:, pr, a:a + 1, :, :].squeeze(2)
                    nc.vector.tensor_tensor(out_a, in1[:, :, :, :],
                                            in2[:, pr, :, :], OP.mult)

            def prod(pair_idx, a, b):
                base = (pair_idx * 4 + a * 2 + b) * J
                return PROD[:, base:base + J]

            PAIR_IDX = {(0, 1): 0, (2, 3): 1, (4, 5): 2, (6, 7): 3}
            PRUNE = float(prune_thr)

            def emit_chain(name, pair, w9):
                """q = sum_a w9[a]*mono_a over pair.  Returns None (zero),
                float (constant) or a tile.  mono a = 3*iA+iB, features
                [1, -s, c] per wire."""
                wA, wB = pair
                pi = PAIR_IDX[pair]
                cand = [
                    (Sw(wB), -w9[1]), (Cw(wB), w9[2]),
                    (Sw(wA), -w9[3]), (Cw(wA), w9[6]),
                    (prod(pi, 0, 0), w9[4]), (prod(pi, 0, 1), -w9[5]),
                    (prod(pi, 1, 0), -w9[7]), (prod(pi, 1, 1), w9[8]),
                ]
                terms = [(ap, c) for (ap, c) in cand if abs(c) > PRUNE]
                if not terms:
                    if abs(w9[0]) <= PRUNE:
                        return None
                    return float(w9[0])
                q = pool.tile([P, J], f32, tag=name)
                ap0, c0 = terms[0]
                nc.vector.tensor_scalar(q[:, :], ap0, float(c0), float(w9[0]),
                                        OP.mult, OP.add)
                for (ap, c) in terms[1:]:
                    nc.vector.scalar_tensor_tensor(q[:, :], ap, float(c),
                                                   q[:, :], OP.mult, OP.add)
                return q

            def emit_side(Wu, Vv, upair, vpair, tag):
                """Returns per-k (acc_tile_or_None, bias) for
                uX_k = sum_m chain(Wu[:,m]) * chain(Vv[m,:,k])."""
                R = Wu.shape[1]
                K = Vv.shape[2]
                us = [emit_chain(f"u{tag}{m}", upair, Wu[:, m])
                      for m in range(R)]
                outs = []
                for k in range(K):
                    merged = np.zeros(9)
                    mpairs = []
                    for m in range(R):
                        vcoef = Vv[m, :, k]
                        if not np.any(np.abs(vcoef) > PRUNE):
                            continue
                        if us[m] is None:
                            continue
                        if isinstance(us[m], float):
                            merged = merged + us[m] * vcoef
                        else:
                            mpairs.append((us[m], vcoef))
                    acc = None
                    bias = 0.0
                    if np.any(np.abs(merged) > PRUNE):
                        mc = emit_chain(f"w{tag}{k}", vpair, merged)
                        if isinstance(mc, float):
                            bias += mc
                        elif mc is not None:
                            acc = mc
                    for i, (ut, vcoef) in enumerate(mpairs):
                        vc = emit_chain(f"v{tag}{k}_{i}", vpair, vcoef)
                        if vc is None:
                            continue
                        if isinstance(vc, float):
                            if acc is None:
                                acc = pool.tile([P, J], f32, tag=f"a{tag}{k}")
                                nc.vector.tensor_scalar(
                                    acc[:, :], ut[:, :], float(vc), 0.0,
                                    OP.mult, OP.add)
                            else:
                                nc.vector.scalar_tensor_tensor(
                                    acc[:, :], ut[:, :], float(vc), acc[:, :],
                                    OP.mult, OP.add)
                        else:
                            if acc is None:
                                acc = pool.tile([P, J], f32, tag=f"a{tag}{k}")
                                nc.vector.tensor_mul(acc[:, :], ut[:, :],
                                                     vc[:, :])
                            else:
                                t = pool.tile([P, J], f32, tag=f"t{tag}{k}")
                                nc.vector.tensor_mul(t[:, :], ut[:, :],
                                                     vc[:, :])
                                nc.vector.tensor_add(acc[:, :], acc[:, :],
                                                     t[:, :])
                    outs.append((acc, bias))
                return outs

            uL = emit_side(F["W01"], F["V23"], (0, 1), (2, 3), "L")
            uR = emit_side(F["W67"], F["V45"], (6, 7), (4, 5), "R")

            # top: out = 0.5 + sum_k uL_k * uR_k  (biases folded in)
            const_out = 0.5
            acc = None
            for (aL, bL), (aR, bR) in zip(uL, uR):
                const_out += bL * bR
                for plane, b in ((aL, bR), (aR, bL)):
                    if plane is not None and abs(b) > 1e-14:
                        if acc is None:
                            acc = pool.tile([P, J], f32, tag="top")
                            nc.vector.tensor_scalar(acc[:, :], plane[:, :],
                                                    float(b), 0.0,
                                                    OP.mult, OP.add)
                        else:
                            nc.vector.scalar_tensor_tensor(
                                acc[:, :], plane[:, :], float(b), acc[:, :],
                                OP.mult, OP.add)
                if aL is not None and aR is not None:
                    if acc is None:
                        acc = pool.tile([P, J], f32, tag="top")
                        nc.vector.tensor_mul(acc[:, :], aL[:, :], aR[:, :])
                    else:
                        t = pool.tile([P, J], f32, tag="topt")
                        nc.vector.tensor_mul(t[:, :], aL[:, :], aR[:, :])
                        nc.vector.tensor_add(acc[:, :], acc[:, :], t[:, :])
            OUT = pool.tile([P, J], f32)
            if acc is None:
                nc.vector.memset(OUT[:, :], float(const_out))
            else:
                nc.scalar.activation(OUT[:, :], acc[:, :], AF.Copy,
                                     bias=float(const_out), scale=1.0)

            yv = y_out.rearrange("(p j) o -> p (j o)", p=P)
            out_engines = [nc.sync, nc.gpsimd, nc.scalar, nc.sync]
            for c in range(4):
                out_engines[c].dma_start(yv[32 * c:32 * (c + 1), :],
                                         OUT[32 * c:32 * (c + 1), :])
    return nc


_PROGRAM_CACHE = {}
LAST_RESULT = None


def kernel(x: np.ndarray, theta: np.ndarray) -> np.ndarray:
    import os
    from concourse.bass_utils import run_bass_kernel_spmd

    x = np.ascontiguousarray(np.asarray(x, dtype=np.float32))
    theta = np.asarray(theta, dtype=np.float32)
    assert x.shape == (B_TOTAL, N_QUBITS), x.shape

    key = theta.tobytes()
    nc = _PROGRAM_CACHE.get(key)
    if nc is None:
        F = _factorize(theta)
        nc = _build_program(F, prune_thr=_pick_prune_thr(F))
        _PROGRAM_CACHE[key] = nc

    shards = [x[i * B_CORE:(i + 1) * B_CORE] for i in range(N_CORES)]
    in_maps = [{"x": s} for s in shards]
    trace = bool(int(os.environ.get("KERNEL_PROFILE", "0")))
    res = run_bass_kernel_spmd(nc, in_maps, list(range(N_CORES)), trace=trace)
    global LAST_RESULT
    LAST_RESULT = res
    out = np.concatenate([res.results[i]["out"] for i in range(N_CORES)], axis=0)
    return out.astype(np.float32, copy=False)
